# revision 11
# baseline (speedup 1.0000x reference)
"""BiMambaTextEncoder Trainium2 kernel (chunk-pipelined).

Sharding: 8 cores = 4 batch x 2 direction; backward handled by reversing the
sequence on the host and flipping conv kernels; final projection decomposed so
the host sums fo/bo halves (no collectives).

This version pipelines the conv stack with the Mamba scan phase: everything is
emitted in NC time-chunks of Q columns.  Conv layer l's chunk c covers columns
[c*Q - 2*l, (c+1)*Q - 2*l) so a layer's chunk depends only on chunks 0..c of
the previous layer (the +-2 halo lands exactly on the previous chunk's end).
Downstream (in_proj, dwconv, x_proj, dt, scan, out_proj) uses layer-2's chunk
ranges.  The Tile scheduler then overlaps chunk c's scan work (Vector-bound)
with chunk c+1's conv work (PE-bound), which removes the ~200us of Vector idle
the phase-sequential version had.

Scan layout per (i-block, n-half, chunk): one [128, 8*261] tensor_tensor_scan;
column n*261 is a boundary column holding dA=0 and dBu=carry-state, restarting
the recurrence per state.  DA/DBU tiles are fully memset once per buffer so
never-written tail columns (chunks narrower than 260) stay finite (a NaN there
would poison the boundary restart via 0*NaN).
"""

from contextlib import ExitStack

import numpy as np

import concourse.bass as bass
from concourse import bacc
import concourse.mybir as mybir
import concourse.tile as tile
from concourse.bass_utils import run_bass_kernel_spmd

F16 = mybir.dt.float16
F32 = mybir.dt.float32
AF = mybir.ActivationFunctionType
OP = mybir.AluOpType

B, L, C, K, DEPTH, V = 4, 1024, 512, 5, 3, 178
DI = 1024           # d_inner
N = 16              # d_state
DCONV = 4
DTR = 32            # dt_rank
NCB = C // 128      # 4 channel blocks
NDB = DI // 128     # 8 d_inner blocks
Q = 256             # chunk width
NC = L // Q         # chunks
EPS = 1e-5

NH = 2              # n halves per block
NPH = N // NH       # 8 states per packed scan
WMAX = Q + 4        # widest chunk (last downstream chunk)
SB = WMAX + 1       # 261: per-state stride in packed scan tiles
WB = NPH * SB       # packed scan width


def _ranges(shift):
    rs = []
    for c in range(NC):
        s = max(0, c * Q - shift)
        e = L if c == NC - 1 else (c + 1) * Q - shift
        rs.append((s, e))
    return rs


def _par(param, cob):
    s = param * NCB + cob
    return slice(s, s + 1)


def build_program():
    nc = bacc.Bacc()

    d_h0 = nc.dram_tensor("h0", [C, L], F16, kind="ExternalInput")
    d_convw = nc.dram_tensor("convw", [DEPTH, NCB, 128, K * NCB, 128], F16,
                             kind="ExternalInput")
    d_cpar = nc.dram_tensor("cpar", [DEPTH, 128, 12], F32, kind="ExternalInput")
    d_inw = nc.dram_tensor("inw", [NCB, 128, 2 * DI], F16, kind="ExternalInput")
    d_mcw = nc.dram_tensor("mcw", [NDB, 128, DCONV * 128], F16,
                           kind="ExternalInput")
    d_mpar = nc.dram_tensor("mpar", [128, 16], F32, kind="ExternalInput")
    d_xw = nc.dram_tensor("xw", [NDB, 128, DTR + 2 * N], F16,
                          kind="ExternalInput")
    d_dtw = nc.dram_tensor("dtw", [NDB, DTR, 128], F16, kind="ExternalInput")
    d_An = nc.dram_tensor("An", [NDB, 128, N], F32, kind="ExternalInput")
    d_Dd = nc.dram_tensor("Dd", [NDB, 128, 128], F16, kind="ExternalInput")
    d_outw = nc.dram_tensor("outw", [NDB, 128, C], F16, kind="ExternalInput")
    d_pw = nc.dram_tensor("pw", [NCB, 128, C], F16, kind="ExternalInput")
    d_ident = nc.dram_tensor("ident", [128, 128], F16, kind="ExternalInput")
    d_part = nc.dram_tensor("part", [C, L], F32, kind="ExternalOutput")
    # DRAM bounce for B/C rows of x_proj output (for broadcast reads)
    d_xbc = nc.dram_tensor("xbc", [2 * N, L], F16)

    CR = [_ranges(2 * l) for l in range(DEPTH)]
    MR = CR[DEPTH - 1]

    with tile.TileContext(nc) as tc, ExitStack() as ctx:
        sing = ctx.enter_context(tc.tile_pool(name="sing", bufs=1))
        wp = ctx.enter_context(tc.tile_pool(name="wp", bufs=1))
        hp = ctx.enter_context(tc.tile_pool(name="hp", bufs=1))
        bcp = ctx.enter_context(tc.tile_pool(name="bcp", bufs=1))
        sp = ctx.enter_context(tc.tile_pool(name="sp", bufs=1))
        st = ctx.enter_context(tc.tile_pool(name="st", bufs=1))
        pp = ctx.enter_context(tc.tile_pool(name="pp", bufs=1, space="PSUM"))

        dma = nc.sync.dma_start

        def T(pool, shape, dt, tag, bufs, name):
            return pool.tile(shape, dt, tag=tag, bufs=bufs, name=name)

        def r3(t):
            return t[:].rearrange("p (n q) -> p n q", n=NPH)

        # ---- constants / params ----
        ident = T(sing, [128, 128], F16, "ident", 1, "ident")
        dma(out=ident[:], in_=d_ident[:])
        ones = T(sing, [128, 1], F16, "ones", 1, "ones")
        nc.vector.memset(ones[:], 1.0)
        ones32 = T(sing, [128, 1], F32, "ones32", 1, "ones32")
        nc.vector.memset(ones32[:], 1.0)
        epst = T(sing, [1, 1], F32, "epst", 1, "epst")
        nc.vector.memset(epst[:], EPS)
        cpar = []
        for l in range(DEPTH):
            t = T(sing, [128, 12], F32, f"cpar{l}", 1, f"cpar{l}")
            dma(out=t[:], in_=d_cpar[l])
            cpar.append(t)
        mpar = T(sing, [128, 16], F32, "mpar", 1, "mpar")
        dma(out=mpar[:], in_=d_mpar[:])
        An = []
        for i in range(NDB):
            t = T(sing, [128, N], F32, f"An{i}", 1, f"An{i}")
            dma(out=t[:], in_=d_An[i])
            An.append(t)
        states = []
        for i in range(NDB):
            t = T(sing, [128, N], F16, f"stt{i}", 1, f"stt{i}")
            states.append(t)

        # pre-touch every ACT-consumed param tile on the scalar engine so the
        # real consumers don't exceed the Activation ISA sync-wait limit
        touch = T(sing, [128, 224], F16, "touch", 1, "touch")
        for ti_, tt_ in enumerate(cpar + [mpar] + An):
            w_ = tt_.shape[-1]
            nc.scalar.copy(out=touch[:, ti_ * 16: ti_ * 16 + w_], in_=tt_[:])
        nc.scalar.copy(out=touch[0:1, 223:224], in_=epst[:])

        LP = L + 4
        hbuf = [[T(hp, [128, LP], F16, "big", 8, f"hbuf{s}_{cb}")
                 for cb in range(NCB)] for s in range(2)]
        for s in range(2):
            for cb in range(NCB):
                nc.vector.memset(hbuf[s][cb][:, 0:2], 0.0)
                nc.vector.memset(hbuf[s][cb][:, L + 2:LP], 0.0)

        for cb in range(NCB):
            dma(out=hbuf[0][cb][:, 2:2 + L],
                in_=d_h0[cb * 128:(cb + 1) * 128, :])

        # ---- resident weights ----
        cw = [[None] * NCB for _ in range(DEPTH)]
        for l in range(DEPTH):
            for cib in range(NCB):
                t = T(wp, [128, K * NCB * 128], F16, "cw", 12, f"cw{l}_{cib}")
                dma(out=t[:], in_=d_convw[l, cib])
                cw[l][cib] = t
        inw = []
        for cib in range(NCB):
            t = T(wp, [128, 2 * DI], F16, "inw", 4, f"inw{cib}")
            dma(out=t[:], in_=d_inw[cib])
            inw.append(t)
        mcw = []
        for i in range(NDB):
            t = T(wp, [128, DCONV * 128], F16, "mcw", 8, f"mcw{i}")
            dma(out=t[:], in_=d_mcw[i])
            mcw.append(t)
        xw = []
        for i in range(NDB):
            t = T(wp, [128, DTR + 2 * N], F16, f"xw{i}", 1, f"xw{i}")
            dma(out=t[:], in_=d_xw[i])
            xw.append(t)
        dtw = []
        for i in range(NDB):
            t = T(wp, [DTR, 128], F16, f"dtw{i}", 1, f"dtw{i}")
            dma(out=t[:], in_=d_dtw[i])
            dtw.append(t)
        Dd = []
        for i in range(NDB):
            t = T(wp, [128, 128], F16, f"Dd{i}", 1, f"Dd{i}")
            dma(out=t[:], in_=d_Dd[i])
            Dd.append(t)
        pw = []
        for cib in range(NCB):
            t = T(wp, [128, C], F16, "pw", 4, f"pw{cib}")
            dma(out=t[:], in_=d_pw[cib])
            pw.append(t)

        xdbc = T(hp, [DTR + 2 * N, L], F16, "xdbc", 1, "xdbc")

        ub_prev = None      # previous chunk's ubuf tiles (for the 3-col halo)
        dbu_zeroed = [False]
        da_zeroed = [0]

        for c in range(NC):
            # ================= conv stack =================
            for l in range(DEPTH):
                s, e = CR[l][c]
                w = e - s
                src = hbuf[l % 2]
                dst = hbuf[(l + 1) % 2]
                craw = [T(st, [128, WMAX], F16, "craw", 6, f"craw{l}_{cob}_{c}")
                        for cob in range(NCB)]
                for cob in range(NCB):
                    ps = T(pp, [128, WMAX], F32, "mm", 2, f"ps_c{l}_{cob}_{c}")
                    first = True
                    for cib in range(NCB):
                        for k in range(K):
                            j = k * NCB + cob
                            nc.tensor.matmul(
                                ps[:, 0:w],
                                cw[l][cib][:, j * 128:(j + 1) * 128],
                                src[cib][:, s + k: s + k + w],
                                start=first,
                                stop=(cib == NCB - 1 and k == K - 1))
                            first = False
                    nc.scalar.activation(
                        out=craw[cob][:, 0:w], in_=ps[:, 0:w],
                        func=AF.Identity, bias=cpar[l][:, _par(0, cob)],
                        scale=1.0)
                ps_s = T(pp, [1, WMAX], F32, "st", 2, f"ps_s{l}_{c}")
                ps_q = T(pp, [1, WMAX], F32, "st", 2, f"ps_q{l}_{c}")
                for cob in range(NCB):
                    nc.tensor.matmul(ps_s[:, 0:w], ones[:],
                                     craw[cob][:, 0:w],
                                     start=(cob == 0), stop=(cob == NCB - 1))
                for cob in range(NCB):
                    sq = T(st, [128, WMAX], F16, "csq", 1, f"csq{l}_{cob}_{c}")
                    nc.scalar.activation(out=sq[:, 0:w],
                                         in_=craw[cob][:, 0:w],
                                         func=AF.Square)
                    nc.tensor.matmul(ps_q[:, 0:w], ones[:], sq[:, 0:w],
                                     start=(cob == 0), stop=(cob == NCB - 1))
                mu = T(st, [1, WMAX], F32, "row", 3, f"mu{l}_{c}")
                nc.vector.tensor_scalar_mul(mu[:, 0:w], ps_s[:, 0:w], 1.0 / C)
                var = T(st, [1, WMAX], F32, "row", 3, f"var{l}_{c}")
                nc.vector.tensor_mul(var[:, 0:w], mu[:, 0:w], mu[:, 0:w])
                nc.vector.tensor_scalar_mul(var[:, 0:w], var[:, 0:w], -1.0)
                nc.vector.scalar_tensor_tensor(
                    out=var[:, 0:w], in0=ps_q[:, 0:w], scalar=1.0 / C,
                    in1=var[:, 0:w], op0=OP.mult, op1=OP.add)
                nc.scalar.activation(out=var[:, 0:w], in_=var[:, 0:w],
                                     func=AF.Sqrt, bias=epst[:], scale=1.0)
                rstd = T(st, [1, WMAX], F32, "row", 3, f"rstd{l}_{c}")
                nc.vector.reciprocal_approx_fast(out=rstd[:, 0:w],
                                                 in_=var[:, 0:w])
                nmr = T(st, [1, WMAX], F16, "row16", 2, f"nmr{l}_{c}")
                nc.vector.tensor_mul(nmr[:, 0:w], mu[:, 0:w], rstd[:, 0:w])
                nc.vector.tensor_scalar_mul(nmr[:, 0:w], nmr[:, 0:w], -1.0)
                rstd16 = T(st, [1, WMAX], F16, "row16", 2, f"rstd16{l}_{c}")
                nc.vector.tensor_copy(out=rstd16[:, 0:w], in_=rstd[:, 0:w])
                rs_bc = T(st, [128, WMAX], F16, "rs_bc", 2, f"rs_bc{l}_{c}")
                nc.gpsimd.partition_broadcast(rs_bc[:, 0:w], rstd16[:, 0:w])
                nm_bc = T(st, [128, WMAX], F16, "nm_bc", 2, f"nm_bc{l}_{c}")
                nc.gpsimd.partition_broadcast(nm_bc[:, 0:w], nmr[:, 0:w])
                for cob in range(NCB):
                    t2 = T(st, [128, WMAX], F16, "lnt", 2, f"lnt{l}_{cob}_{c}")
                    nc.vector.tensor_mul(t2[:, 0:w], craw[cob][:, 0:w],
                                         rs_bc[:, 0:w])
                    nc.vector.tensor_add(t2[:, 0:w], t2[:, 0:w],
                                         nm_bc[:, 0:w])
                    nc.scalar.activation(
                        out=dst[cob][:, 2 + s: 2 + e],
                        in_=t2[:, 0:w], func=AF.Prelu,
                        bias=cpar[l][:, _par(2, cob)],
                        scale=cpar[l][:, _par(1, cob)], alpha=0.2)

            hfin = hbuf[DEPTH % 2]
            s, e = MR[c]
            w = e - s

            # ================= in_proj (u half) =================
            ub = [T(hp, [128, 3 + WMAX], F16, "ub", 12, f"ub{i}_{c}")
                  for i in range(NDB)]
            for i in range(NDB):
                if c == 0:
                    nc.vector.memset(ub[i][:, 0:3], 0.0)
                else:
                    pw_ = MR[c - 1][1] - MR[c - 1][0]
                    nc.scalar.copy(out=ub[i][:, 0:3],
                                   in_=ub_prev[i][:, pw_:pw_ + 3])
            for m in range(NDB):
                ps = T(pp, [128, WMAX], F32, "mm", 2, f"ps_in{m}_{c}")
                for cib in range(NCB):
                    nc.tensor.matmul(
                        ps[:, 0:w], inw[cib][:, m * 128:(m + 1) * 128],
                        hfin[cib][:, 2 + s: 2 + e],
                        start=(cib == 0), stop=(cib == NCB - 1))
                nc.scalar.copy(out=ub[m][:, 3:3 + w], in_=ps[:, 0:w])

            # ================= causal depthwise conv + SiLU =================
            uc = [T(hp, [128, WMAX], F16, "uc", 12, f"uc{i}_{c}")
                  for i in range(NDB)]
            for i in range(NDB):
                ps = T(pp, [128, WMAX], F32, "mm", 2, f"ps_mc{i}_{c}")
                for k in range(DCONV):
                    nc.tensor.matmul(
                        ps[:, 0:w], mcw[i][:, k * 128:(k + 1) * 128],
                        ub[i][:, k: k + w],
                        start=(k == 0), stop=(k == DCONV - 1))
                nc.scalar.activation(
                    out=uc[i][:, 0:w], in_=ps[:, 0:w],
                    func=AF.Silu, bias=mpar[:, i:i + 1], scale=1.0)

            # ================= x_proj =================
            ps = T(pp, [DTR + 2 * N, WMAX], F32, "mm", 2, f"ps_x{c}")
            for i in range(NDB):
                nc.tensor.matmul(ps[:, 0:w], xw[i][:], uc[i][:, 0:w],
                                 start=(i == 0), stop=(i == NDB - 1))
            nc.scalar.copy(out=xdbc[:, s:e], in_=ps[:, 0:w])
            dma(out=d_xbc[:, s:e], in_=xdbc[DTR:DTR + 2 * N, s:e])

            # ================= in_proj (z half) -> silz =================
            sz = [T(hp, [128, WMAX], F16, "sz", 12, f"sz{i}_{c}")
                  for i in range(NDB)]
            for m in range(NDB, 2 * NDB):
                ps = T(pp, [128, WMAX], F32, "mm", 2, f"ps_in{m}_{c}")
                for cib in range(NCB):
                    nc.tensor.matmul(
                        ps[:, 0:w], inw[cib][:, m * 128:(m + 1) * 128],
                        hfin[cib][:, 2 + s: 2 + e],
                        start=(cib == 0), stop=(cib == NCB - 1))
                nc.scalar.activation(out=sz[m - NDB][:, 0:w], in_=ps[:, 0:w],
                                     func=AF.Silu)

            # ================= dt_proj -> softplus =================
            deltas = []
            for i in range(NDB):
                ps = T(pp, [128, WMAX], F32, "mm", 2, f"ps_dt{i}_{c}")
                nc.tensor.matmul(ps[:, 0:w], dtw[i][:], xdbc[0:DTR, s:e],
                                 start=True, stop=True)
                dl = T(sp, [128, WMAX], F16, "delta", 10, f"delta{i}_{c}")
                nc.scalar.activation(out=dl[:, 0:w], in_=ps[:, 0:w],
                                     func=AF.Exp,
                                     bias=mpar[:, 8 + i:9 + i], scale=1.0)
                deltas.append(dl)
            for i in range(NDB):
                nc.scalar.activation(out=deltas[i][:, 0:w],
                                     in_=deltas[i][:, 0:w],
                                     func=AF.Ln, bias=ones32[:], scale=1.0)

            # ================= B/C broadcast tiles =================
            ball, call_ = [], []
            for h in range(NH):
                bt = T(bcp, [128, WB], F16, "ball", 2, f"ball{c}_{h}")
                for p0 in (0, 64):
                    dma(out=r3(bt)[p0:p0 + 64, :, 1:w + 1],
                        in_=d_xbc[NPH * h:NPH * h + NPH,
                                  s:e].partition_broadcast(64))
                ball.append(bt)
                ct = T(bcp, [128, WB], F16, "call", 2, f"call{c}_{h}")
                for p0 in (0, 64):
                    dma(out=r3(ct)[p0:p0 + 64, :, 1:w + 1],
                        in_=d_xbc[N + NPH * h:N + NPH * h + NPH,
                                  s:e].partition_broadcast(64))
                call_.append(ct)

            # ================= scan blocks =================
            yfin = [T(hp, [128, WMAX], F16, "yf", 10, f"yf{i}_{c}")
                    for i in range(NDB)]
            pend = []
            for i in range(NDB):
                yp = T(pp, [128, WMAX], F32, "y", 2, f"yp{i}_{c}")
                nc.tensor.matmul(yp[:, 0:w], Dd[i][:], uc[i][:, 0:w],
                                 start=True, stop=False)
                du = T(sp, [128, WMAX], F16, "du", 2, f"du{i}_{c}")
                nc.vector.tensor_mul(du[:, 0:w], deltas[i][:, 0:w],
                                     uc[i][:, 0:w])
                if pend:
                    pi, pyp, wq = pend.pop()
                    yq = T(sp, [128, WMAX], F16, "yq", 2, f"yq{pi}_{c}")
                    nc.scalar.copy(out=yq[:, 0:wq], in_=pyp[:, 0:wq])
                    nc.vector.tensor_mul(yfin[pi][:, 0:wq], yq[:, 0:wq],
                                         sz[pi][:, 0:wq])
                for h in range(NH):
                    da = T(sp, [128, WB], F16, "DA", 2, f"da{c}_{i}_{h}")
                    da3 = r3(da)
                    if da_zeroed[0] < 2:
                        # full-tile zero once per buffer: boundary cols stay 0
                        # and never-written tail cols stay finite
                        nc.vector.memset(da[:], 0.0)
                        da_zeroed[0] += 1
                    for nl in range(NPH):
                        n = h * NPH + nl
                        nc.scalar.activation(
                            out=da3[:, nl:nl + 1, 1:w + 1],
                            in_=deltas[i][:, 0:w], func=AF.Exp,
                            scale=An[i][:, n:n + 1])
                    dbu = T(sp, [128, WB], F16, "DBU", 1, f"dbu{c}_{i}_{h}")
                    dbu3 = r3(dbu)
                    if not dbu_zeroed[0]:
                        nc.vector.memset(dbu[:], 0.0)
                        dbu_zeroed[0] = True
                    if c > 0:
                        nc.vector.tensor_copy(
                            out=dbu3[:, :, 0:1],
                            in_=states[i][:, h * NPH:(h + 1) * NPH])
                    du_b = du[:, 0:w].unsqueeze(1).broadcast_to([128, NPH, w])
                    nc.vector.tensor_tensor(out=dbu3[:, :, 1:w + 1],
                                            in0=du_b,
                                            in1=r3(ball[h])[:, :, 1:w + 1],
                                            op=OP.mult)
                    ht = T(sp, [128, WB], F16, "H", 2, f"h{c}_{i}_{h}")
                    nc.vector.tensor_tensor_scan(ht[:], da[:], dbu[:], 0.0,
                                                 OP.mult, OP.add)
                    h3 = r3(ht)
                    if c < NC - 1:
                        nc.vector.tensor_copy(
                            out=states[i][:, h * NPH:(h + 1) * NPH],
                            in_=h3[:, :, w:w + 1])
                    hc = T(sp, [128, NPH * WMAX], F16, "HC", 2,
                           f"hc{c}_{i}_{h}")
                    hc3 = hc[:].rearrange("p (n q) -> p n q", n=NPH)
                    nc.vector.tensor_tensor(out=hc3[:, :, 0:w],
                                            in0=h3[:, :, 1:w + 1],
                                            in1=r3(call_[h])[:, :, 1:w + 1],
                                            op=OP.mult)
                    for nl in range(NPH):
                        nc.tensor.matmul(yp[:, 0:w], ident[:],
                                         hc[:, nl * WMAX: nl * WMAX + w],
                                         start=False,
                                         stop=(h == NH - 1 and nl == NPH - 1))
                pend.append((i, yp, w))
            pi, pyp, wq = pend.pop()
            yq = T(sp, [128, WMAX], F16, "yq", 2, f"yq{pi}_{c}")
            nc.scalar.copy(out=yq[:, 0:wq], in_=pyp[:, 0:wq])
            nc.vector.tensor_mul(yfin[pi][:, 0:wq], yq[:, 0:wq],
                                 sz[pi][:, 0:wq])

            # ================= out_proj + final proj =================
            yo = [T(st, [128, WMAX], F16, "yo", 4, f"yo{cb}_{c}")
                  for cb in range(NCB)]
            for cb in range(NCB):
                ps_o = T(pp, [128, WMAX], F32, "dtm", 2, f"ps_o{cb}_{c}")
                for wv in range(2):
                    oww = []
                    for j in range(4):
                        i = wv * 4 + j
                        w_ = T(st, [128, C], F16, "ow", 2, f"outw{c}_{cb}_{i}")
                        dma(out=w_[:], in_=d_outw[i])
                        oww.append(w_)
                    for j in range(4):
                        i = wv * 4 + j
                        nc.tensor.matmul(
                            ps_o[:, 0:w], oww[j][:, cb * 128:(cb + 1) * 128],
                            yfin[i][:, 0:w],
                            start=(i == 0), stop=(i == NDB - 1))
                nc.scalar.copy(out=yo[cb][:, 0:w], in_=ps_o[:, 0:w])
            for cb in range(NCB):
                ps_p = T(pp, [128, WMAX], F32, "mm", 2, f"ps_p{cb}_{c}")
                for cib in range(NCB):
                    nc.tensor.matmul(ps_p[:, 0:w],
                                     pw[cib][:, cb * 128:(cb + 1) * 128],
                                     yo[cib][:, 0:w],
                                     start=(cib == 0), stop=(cib == NCB - 1))
                ot = T(hp, [128, WMAX], F32, "osb", 2, f"osb{cb}_{c}")
                nc.scalar.copy(out=ot[:, 0:w], in_=ps_p[:, 0:w])
                dma(out=d_part[cb * 128:(cb + 1) * 128, s:e],
                    in_=ot[:, 0:w])

            ub_prev = ub

    nc.compile()
    return nc


_cache = {}


def _prep_core_inputs(inputs, core):
    b = core >> 1
    rev = (core & 1) == 1
    p = "b_" if rev else "f_"
    f16 = np.float16
    f32 = np.float32

    toks = np.asarray(inputs["x"][b]).astype(np.int64)
    if rev:
        toks = toks[::-1]
    embf = np.asarray(inputs["emb"]).astype(f16)
    h0 = np.ascontiguousarray(embf[toks].T)

    key = ("wts", p)
    if key not in _cache:

        cw = np.asarray(inputs["conv_w"]).astype(f32)  # [D, cout, cin, K]
        if rev:
            cw = cw[:, :, :, ::-1]
        convw = np.empty((DEPTH, NCB, 128, K * NCB, 128), f16)
        for l in range(DEPTH):
            for cib in range(NCB):
                for k in range(K):
                    for cob in range(NCB):
                        blk = cw[l, cob * 128:(cob + 1) * 128,
                                 cib * 128:(cib + 1) * 128, k]
                        convw[l, cib, :, k * NCB + cob, :] = blk.T.astype(f16)
        cpar = np.zeros((DEPTH, 128, 12), f32)
        for l in range(DEPTH):
            for cob in range(NCB):
                cs = slice(cob * 128, (cob + 1) * 128)
                cpar[l, :, 0 * NCB + cob] = inputs["conv_b"][l][cs]
                cpar[l, :, 1 * NCB + cob] = inputs["ln_g"][l][cs]
                cpar[l, :, 2 * NCB + cob] = inputs["ln_b"][l][cs]

        in_w = np.asarray(inputs[p + "in_w"]).astype(f32)  # [2*DI, C]
        inw = np.empty((NCB, 128, 2 * DI), f16)
        for cib in range(NCB):
            inw[cib] = in_w[:, cib * 128:(cib + 1) * 128].T.astype(f16)

        mconv = np.asarray(inputs[p + "conv_w"]).astype(f32)  # [DI, 4]
        mcw = np.zeros((NDB, 128, DCONV * 128), f16)
        dd = np.arange(128)
        for i in range(NDB):
            for k in range(DCONV):
                mcw[i, dd, k * 128 + dd] = mconv[i * 128:(i + 1) * 128, k]

        mpar = np.zeros((128, 16), f32)
        for i in range(NDB):
            mpar[:, i] = inputs[p + "conv_b"][i * 128:(i + 1) * 128]
            mpar[:, 8 + i] = inputs[p + "dt_b"][i * 128:(i + 1) * 128]

        x_w = np.asarray(inputs[p + "x_w"]).astype(f32)  # [64, DI]
        xw = np.empty((NDB, 128, DTR + 2 * N), f16)
        for i in range(NDB):
            xw[i] = x_w[:, i * 128:(i + 1) * 128].T.astype(f16)

        dt_w = np.asarray(inputs[p + "dt_w"]).astype(f32)  # [DI, DTR]
        dtw = np.empty((NDB, DTR, 128), f16)
        for i in range(NDB):
            dtw[i] = dt_w[i * 128:(i + 1) * 128, :].T.astype(f16)

        An = (-np.exp(np.asarray(inputs[p + "A_log"]).astype(f32))
              ).reshape(NDB, 128, N).astype(f32)

        Dv = np.asarray(inputs[p + "D"]).astype(f32)
        Dd = np.zeros((NDB, 128, 128), f16)
        for i in range(NDB):
            Dd[i, dd, dd] = Dv[i * 128:(i + 1) * 128]

        out_w = np.asarray(inputs[p + "out_w"]).astype(f32)  # [C, DI]
        outw = np.empty((NDB, 128, C), f16)
        for i in range(NDB):
            outw[i] = out_w[:, i * 128:(i + 1) * 128].T.astype(f16)

        proj_w = np.asarray(inputs["proj_w"]).astype(f32)  # [C, 2C]
        half = proj_w[:, C:] if rev else proj_w[:, :C]
        pw = np.empty((NCB, 128, C), f16)
        for cib in range(NCB):
            pw[cib] = half[:, cib * 128:(cib + 1) * 128].T.astype(f16)

        _cache[key] = dict(
            convw=convw, cpar=cpar, inw=inw, mcw=mcw, mpar=mpar,
            xw=xw, dtw=dtw, An=An, Dd=Dd, outw=outw, pw=pw,
            ident=np.eye(128, dtype=f16))
    m = dict(_cache[key])
    m["h0"] = h0
    return m


def kernel(**inputs):
    if "nc" not in _cache:
        _cache["nc"] = build_program()
    nc = _cache["nc"]
    for k in [k for k in _cache if k != "nc"]:
        del _cache[k]
    in_maps = [_prep_core_inputs(inputs, c) for c in range(8)]
    res = run_bass_kernel_spmd(nc, in_maps, list(range(8)))
    parts = [r["part"] for r in res.results]
    proj_b = np.asarray(inputs["proj_b"]).astype(np.float32)
    out = np.empty((B, L, C), np.float32)
    for b in range(B):
        # note: the reference concatenates bo still in reversed time order
        comb = parts[2 * b] + parts[2 * b + 1]
        out[b] = comb.T + proj_b[None, :]
    m = np.asarray(inputs["m"])
    out = np.where(m[:, :, None], 0.0, out).astype(np.float32)
    return out


# revision 12
# speedup vs baseline: 1.2485x; 1.2485x over previous
"""BiMambaTextEncoder Trainium2 kernel (chunk-pipelined).

Sharding: 8 cores = 4 batch x 2 direction; backward handled by reversing the
sequence on the host and flipping conv kernels; final projection decomposed so
the host sums fo/bo halves (no collectives).

This version pipelines the conv stack with the Mamba scan phase: everything is
emitted in NC time-chunks of Q columns.  Conv layer l's chunk c covers columns
[c*Q - 2*l, (c+1)*Q - 2*l) so a layer's chunk depends only on chunks 0..c of
the previous layer (the +-2 halo lands exactly on the previous chunk's end).
Downstream (in_proj, dwconv, x_proj, dt, scan, out_proj) uses layer-2's chunk
ranges.  The Tile scheduler then overlaps chunk c's scan work (Vector-bound)
with chunk c+1's conv work (PE-bound), which removes the ~200us of Vector idle
the phase-sequential version had.

Scan layout per (i-block, n-half, chunk): one [128, 8*261] tensor_tensor_scan;
column n*261 is a boundary column holding dA=0 and dBu=carry-state, restarting
the recurrence per state.  DA/DBU tiles are fully memset once per buffer so
never-written tail columns (chunks narrower than 260) stay finite (a NaN there
would poison the boundary restart via 0*NaN).
"""

from contextlib import ExitStack

import numpy as np

import concourse.bass as bass
from concourse import bacc
import concourse.mybir as mybir
import concourse.tile as tile
from concourse.bass_utils import run_bass_kernel_spmd

F16 = mybir.dt.float16
F32 = mybir.dt.float32
AF = mybir.ActivationFunctionType
OP = mybir.AluOpType

B, L, C, K, DEPTH, V = 4, 1024, 512, 5, 3, 178
DI = 1024           # d_inner
N = 16              # d_state
DCONV = 4
DTR = 32            # dt_rank
NCB = C // 128      # 4 channel blocks
NDB = DI // 128     # 8 d_inner blocks
Q = 256             # chunk width
NC = L // Q         # chunks
EPS = 1e-5

NH = 2              # n halves per block
NPH = N // NH       # 8 states per packed scan
WMAX = Q + 4        # widest chunk (last downstream chunk)
SB = WMAX + 1       # 261: per-state stride in packed scan tiles
WB = NPH * SB       # packed scan width


def _ranges(shift):
    rs = []
    for c in range(NC):
        s = max(0, c * Q - shift)
        e = L if c == NC - 1 else (c + 1) * Q - shift
        rs.append((s, e))
    return rs


def _par(param, cob):
    s = param * NCB + cob
    return slice(s, s + 1)


def build_program():
    nc = bacc.Bacc()

    d_h0 = nc.dram_tensor("h0", [C, L], F16, kind="ExternalInput")
    d_convw = nc.dram_tensor("convw", [DEPTH, NCB, 128, K * NCB, 128], F16,
                             kind="ExternalInput")
    d_cpar = nc.dram_tensor("cpar", [DEPTH, 128, 12], F32, kind="ExternalInput")
    d_inw = nc.dram_tensor("inw", [NCB, 128, 2 * DI], F16, kind="ExternalInput")
    d_mcw = nc.dram_tensor("mcw", [NDB, 128, DCONV * 128], F16,
                           kind="ExternalInput")
    d_mpar = nc.dram_tensor("mpar", [128, 16], F32, kind="ExternalInput")
    d_xw = nc.dram_tensor("xw", [NDB, 128, DTR + 2 * N], F16,
                          kind="ExternalInput")
    d_dtw = nc.dram_tensor("dtw", [NDB, DTR, 128], F16, kind="ExternalInput")
    d_An = nc.dram_tensor("An", [NDB, 128, N], F32, kind="ExternalInput")
    d_Dd = nc.dram_tensor("Dd", [NDB, 128, 128], F16, kind="ExternalInput")
    d_outw = nc.dram_tensor("outw", [NDB, 128, C], F16, kind="ExternalInput")
    d_pw = nc.dram_tensor("pw", [NCB, 128, C], F16, kind="ExternalInput")
    d_ident = nc.dram_tensor("ident", [128, 128], F16, kind="ExternalInput")
    d_part = nc.dram_tensor("part", [C, L], F32, kind="ExternalOutput")
    # DRAM bounce for B/C rows of x_proj output (for broadcast reads)
    d_xbc = nc.dram_tensor("xbc", [2 * N, L], F16)

    CR = [_ranges(2 * l) for l in range(DEPTH)]
    MR = CR[DEPTH - 1]

    with tile.TileContext(nc) as tc, ExitStack() as ctx:
        sing = ctx.enter_context(tc.tile_pool(name="sing", bufs=1))
        wp = ctx.enter_context(tc.tile_pool(name="wp", bufs=1))
        hp = ctx.enter_context(tc.tile_pool(name="hp", bufs=1))
        bcp = ctx.enter_context(tc.tile_pool(name="bcp", bufs=1))
        sp = ctx.enter_context(tc.tile_pool(name="sp", bufs=1))
        st = ctx.enter_context(tc.tile_pool(name="st", bufs=1))
        pp = ctx.enter_context(tc.tile_pool(name="pp", bufs=1, space="PSUM"))

        dma = nc.sync.dma_start

        def T(pool, shape, dt, tag, bufs, name):
            return pool.tile(shape, dt, tag=tag, bufs=bufs, name=name)

        def r3(t):
            return t[:].rearrange("p (n q) -> p n q", n=NPH)

        # ---- constants / params ----
        ident = T(sing, [128, 128], F16, "ident", 1, "ident")
        dma(out=ident[:], in_=d_ident[:])
        ones = T(sing, [128, 1], F16, "ones", 1, "ones")
        nc.vector.memset(ones[:], 1.0)
        ones32 = T(sing, [128, 1], F32, "ones32", 1, "ones32")
        nc.vector.memset(ones32[:], 1.0)
        epst = T(sing, [1, 1], F32, "epst", 1, "epst")
        nc.vector.memset(epst[:], EPS)
        cpar = []
        for l in range(DEPTH):
            t = T(sing, [128, 12], F32, f"cpar{l}", 1, f"cpar{l}")
            dma(out=t[:], in_=d_cpar[l])
            cpar.append(t)
        mpar = T(sing, [128, 16], F32, "mpar", 1, "mpar")
        dma(out=mpar[:], in_=d_mpar[:])
        An = []
        for i in range(NDB):
            t = T(sing, [128, N], F32, f"An{i}", 1, f"An{i}")
            dma(out=t[:], in_=d_An[i])
            An.append(t)
        states = []
        for i in range(NDB):
            t = T(sing, [128, N], F16, f"stt{i}", 1, f"stt{i}")
            states.append(t)

        # pre-touch every ACT-consumed param tile on the scalar engine so the
        # real consumers don't exceed the Activation ISA sync-wait limit
        touch = T(sing, [128, 224], F16, "touch", 1, "touch")
        for ti_, tt_ in enumerate(cpar + [mpar] + An):
            w_ = tt_.shape[-1]
            nc.scalar.copy(out=touch[:, ti_ * 16: ti_ * 16 + w_], in_=tt_[:])
        nc.scalar.copy(out=touch[0:1, 223:224], in_=epst[:])

        LP = L + 4
        hbuf = [[T(hp, [128, LP], F16, "big", 8, f"hbuf{s}_{cb}")
                 for cb in range(NCB)] for s in range(2)]
        for s in range(2):
            for cb in range(NCB):
                nc.vector.memset(hbuf[s][cb][:, 0:2], 0.0)
                nc.vector.memset(hbuf[s][cb][:, L + 2:LP], 0.0)

        for cb in range(NCB):
            dma(out=hbuf[0][cb][:, 2:2 + L],
                in_=d_h0[cb * 128:(cb + 1) * 128, :])

        # ---- resident weights ----
        cw = [[None] * NCB for _ in range(DEPTH)]
        for l in range(DEPTH):
            for cib in range(NCB):
                t = T(wp, [128, K * NCB * 128], F16, "cw", 12, f"cw{l}_{cib}")
                dma(out=t[:], in_=d_convw[l, cib])
                cw[l][cib] = t
        inw = []
        for cib in range(NCB):
            t = T(wp, [128, 2 * DI], F16, "inw", 4, f"inw{cib}")
            dma(out=t[:], in_=d_inw[cib])
            inw.append(t)
        mcw = []
        for i in range(NDB):
            t = T(wp, [128, DCONV * 128], F16, "mcw", 8, f"mcw{i}")
            dma(out=t[:], in_=d_mcw[i])
            mcw.append(t)
        xw = []
        for i in range(NDB):
            t = T(wp, [128, DTR + 2 * N], F16, f"xw{i}", 1, f"xw{i}")
            dma(out=t[:], in_=d_xw[i])
            xw.append(t)
        dtw = []
        for i in range(NDB):
            t = T(wp, [DTR, 128], F16, f"dtw{i}", 1, f"dtw{i}")
            dma(out=t[:], in_=d_dtw[i])
            dtw.append(t)
        Dd = []
        for i in range(NDB):
            t = T(wp, [128, 128], F16, f"Dd{i}", 1, f"Dd{i}")
            dma(out=t[:], in_=d_Dd[i])
            Dd.append(t)
        pw = []
        for cib in range(NCB):
            t = T(wp, [128, C], F16, "pw", 4, f"pw{cib}")
            dma(out=t[:], in_=d_pw[cib])
            pw.append(t)

        xdbc = T(hp, [DTR + 2 * N, L], F16, "xdbc", 1, "xdbc")

        ub_prev = None      # previous chunk's ubuf tiles (for the 3-col halo)
        dbu_zeroed = [False]
        da_zeroed = [0]

        for c in range(NC):
            # ================= conv stack =================
            for l in range(DEPTH):
                s, e = CR[l][c]
                w = e - s
                src = hbuf[l % 2]
                dst = hbuf[(l + 1) % 2]
                craw = [T(st, [128, WMAX], F16, "craw", 6, f"craw{l}_{cob}_{c}")
                        for cob in range(NCB)]
                for cob in range(NCB):
                    ps = T(pp, [128, WMAX], F32, "mm", 2, f"ps_c{l}_{cob}_{c}")
                    first = True
                    for cib in range(NCB):
                        for k in range(K):
                            j = k * NCB + cob
                            nc.tensor.matmul(
                                ps[:, 0:w],
                                cw[l][cib][:, j * 128:(j + 1) * 128],
                                src[cib][:, s + k: s + k + w],
                                start=first,
                                stop=(cib == NCB - 1 and k == K - 1))
                            first = False
                    nc.scalar.activation(
                        out=craw[cob][:, 0:w], in_=ps[:, 0:w],
                        func=AF.Identity, bias=cpar[l][:, _par(0, cob)],
                        scale=1.0)
                ps_s = T(pp, [1, WMAX], F32, "st", 2, f"ps_s{l}_{c}")
                ps_q = T(pp, [1, WMAX], F32, "st", 2, f"ps_q{l}_{c}")
                for cob in range(NCB):
                    nc.tensor.matmul(ps_s[:, 0:w], ones[:],
                                     craw[cob][:, 0:w],
                                     start=(cob == 0), stop=(cob == NCB - 1))
                for cob in range(NCB):
                    sq = T(st, [128, WMAX], F16, "csq", 1, f"csq{l}_{cob}_{c}")
                    nc.scalar.activation(out=sq[:, 0:w],
                                         in_=craw[cob][:, 0:w],
                                         func=AF.Square)
                    nc.tensor.matmul(ps_q[:, 0:w], ones[:], sq[:, 0:w],
                                     start=(cob == 0), stop=(cob == NCB - 1))
                mu = T(st, [1, WMAX], F32, "row", 3, f"mu{l}_{c}")
                nc.vector.tensor_scalar_mul(mu[:, 0:w], ps_s[:, 0:w], 1.0 / C)
                var = T(st, [1, WMAX], F32, "row", 3, f"var{l}_{c}")
                nc.vector.tensor_mul(var[:, 0:w], mu[:, 0:w], mu[:, 0:w])
                nc.vector.tensor_scalar_mul(var[:, 0:w], var[:, 0:w], -1.0)
                nc.vector.scalar_tensor_tensor(
                    out=var[:, 0:w], in0=ps_q[:, 0:w], scalar=1.0 / C,
                    in1=var[:, 0:w], op0=OP.mult, op1=OP.add)
                nc.scalar.activation(out=var[:, 0:w], in_=var[:, 0:w],
                                     func=AF.Sqrt, bias=epst[:], scale=1.0)
                rstd = T(st, [1, WMAX], F32, "row", 3, f"rstd{l}_{c}")
                nc.vector.reciprocal_approx_fast(out=rstd[:, 0:w],
                                                 in_=var[:, 0:w])
                nmr = T(st, [1, WMAX], F16, "row16", 2, f"nmr{l}_{c}")
                nc.vector.tensor_mul(nmr[:, 0:w], mu[:, 0:w], rstd[:, 0:w])
                nc.vector.tensor_scalar_mul(nmr[:, 0:w], nmr[:, 0:w], -1.0)
                rstd16 = T(st, [1, WMAX], F16, "row16", 2, f"rstd16{l}_{c}")
                nc.vector.tensor_copy(out=rstd16[:, 0:w], in_=rstd[:, 0:w])
                rs_bc = T(st, [128, WMAX], F16, "rs_bc", 2, f"rs_bc{l}_{c}")
                nc.gpsimd.partition_broadcast(rs_bc[:, 0:w], rstd16[:, 0:w])
                nm_bc = T(st, [128, WMAX], F16, "nm_bc", 2, f"nm_bc{l}_{c}")
                nc.gpsimd.partition_broadcast(nm_bc[:, 0:w], nmr[:, 0:w])
                for cob in range(NCB):
                    t2 = T(st, [128, WMAX], F16, "lnt", 2, f"lnt{l}_{cob}_{c}")
                    nc.vector.tensor_mul(t2[:, 0:w], craw[cob][:, 0:w],
                                         rs_bc[:, 0:w])
                    nc.vector.tensor_add(t2[:, 0:w], t2[:, 0:w],
                                         nm_bc[:, 0:w])
                    nc.scalar.activation(
                        out=dst[cob][:, 2 + s: 2 + e],
                        in_=t2[:, 0:w], func=AF.Prelu,
                        bias=cpar[l][:, _par(2, cob)],
                        scale=cpar[l][:, _par(1, cob)], alpha=0.2)

            hfin = hbuf[DEPTH % 2]
            s, e = MR[c]
            w = e - s

            # ================= in_proj (u half) =================
            ub = [T(hp, [128, 3 + WMAX], F16, "ub", 12, f"ub{i}_{c}")
                  for i in range(NDB)]
            for i in range(NDB):
                if c == 0:
                    nc.vector.memset(ub[i][:, 0:3], 0.0)
                else:
                    pw_ = MR[c - 1][1] - MR[c - 1][0]
                    nc.scalar.copy(out=ub[i][:, 0:3],
                                   in_=ub_prev[i][:, pw_:pw_ + 3])
            for m in range(NDB):
                ps = T(pp, [128, WMAX], F32, "mp", 2, f"ps_in{m}_{c}")
                for cib in range(NCB):
                    nc.tensor.matmul(
                        ps[:, 0:w], inw[cib][:, m * 128:(m + 1) * 128],
                        hfin[cib][:, 2 + s: 2 + e],
                        start=(cib == 0), stop=(cib == NCB - 1))
                nc.scalar.copy(out=ub[m][:, 3:3 + w], in_=ps[:, 0:w])

            # ================= causal depthwise conv + SiLU =================
            uc = [T(hp, [128, WMAX], F16, "uc", 12, f"uc{i}_{c}")
                  for i in range(NDB)]
            for i in range(NDB):
                ps = T(pp, [128, WMAX], F32, "mp", 2, f"ps_mc{i}_{c}")
                for k in range(DCONV):
                    nc.tensor.matmul(
                        ps[:, 0:w], mcw[i][:, k * 128:(k + 1) * 128],
                        ub[i][:, k: k + w],
                        start=(k == 0), stop=(k == DCONV - 1))
                nc.scalar.activation(
                    out=uc[i][:, 0:w], in_=ps[:, 0:w],
                    func=AF.Silu, bias=mpar[:, i:i + 1], scale=1.0)

            # ================= x_proj =================
            ps = T(pp, [DTR + 2 * N, WMAX], F32, "mp", 2, f"ps_x{c}")
            for i in range(NDB):
                nc.tensor.matmul(ps[:, 0:w], xw[i][:], uc[i][:, 0:w],
                                 start=(i == 0), stop=(i == NDB - 1))
            nc.scalar.copy(out=xdbc[:, s:e], in_=ps[:, 0:w])
            dma(out=d_xbc[:, s:e], in_=xdbc[DTR:DTR + 2 * N, s:e])

            # ================= in_proj (z half) -> silz =================
            sz = [T(hp, [128, WMAX], F16, "sz", 12, f"sz{i}_{c}")
                  for i in range(NDB)]
            for m in range(NDB, 2 * NDB):
                ps = T(pp, [128, WMAX], F32, "mp", 2, f"ps_in{m}_{c}")
                for cib in range(NCB):
                    nc.tensor.matmul(
                        ps[:, 0:w], inw[cib][:, m * 128:(m + 1) * 128],
                        hfin[cib][:, 2 + s: 2 + e],
                        start=(cib == 0), stop=(cib == NCB - 1))
                nc.scalar.activation(out=sz[m - NDB][:, 0:w], in_=ps[:, 0:w],
                                     func=AF.Silu)

            # ================= dt_proj -> softplus =================
            deltas = []
            for i in range(NDB):
                ps = T(pp, [128, WMAX], F32, "mp", 2, f"ps_dt{i}_{c}")
                nc.tensor.matmul(ps[:, 0:w], dtw[i][:], xdbc[0:DTR, s:e],
                                 start=True, stop=True)
                dl = T(sp, [128, WMAX], F16, "delta", 10, f"delta{i}_{c}")
                nc.scalar.activation(out=dl[:, 0:w], in_=ps[:, 0:w],
                                     func=AF.Exp,
                                     bias=mpar[:, 8 + i:9 + i], scale=1.0)
                deltas.append(dl)
            for i in range(NDB):
                nc.scalar.activation(out=deltas[i][:, 0:w],
                                     in_=deltas[i][:, 0:w],
                                     func=AF.Ln, bias=ones32[:], scale=1.0)

            # ================= B/C broadcast tiles =================
            ball, call_ = [], []
            for h in range(NH):
                bt = T(bcp, [128, WB], F16, "ball", 2, f"ball{c}_{h}")
                for p0 in (0, 64):
                    dma(out=r3(bt)[p0:p0 + 64, :, 1:w + 1],
                        in_=d_xbc[NPH * h:NPH * h + NPH,
                                  s:e].partition_broadcast(64))
                ball.append(bt)
                ct = T(bcp, [128, WB], F16, "call", 2, f"call{c}_{h}")
                for p0 in (0, 64):
                    dma(out=r3(ct)[p0:p0 + 64, :, 1:w + 1],
                        in_=d_xbc[N + NPH * h:N + NPH * h + NPH,
                                  s:e].partition_broadcast(64))
                call_.append(ct)

            # ================= scan blocks =================
            yfin = [T(hp, [128, WMAX], F16, "yf", 10, f"yf{i}_{c}")
                    for i in range(NDB)]
            pend = []
            for i in range(NDB):
                yp = T(pp, [128, WMAX], F32, "y", 2, f"yp{i}_{c}")
                nc.tensor.matmul(yp[:, 0:w], Dd[i][:], uc[i][:, 0:w],
                                 start=True, stop=False)
                du = T(sp, [128, WMAX], F16, "du", 2, f"du{i}_{c}")
                nc.vector.tensor_mul(du[:, 0:w], deltas[i][:, 0:w],
                                     uc[i][:, 0:w])
                if pend:
                    pi, pyp, wq = pend.pop()
                    yq = T(sp, [128, WMAX], F16, "yq", 2, f"yq{pi}_{c}")
                    nc.scalar.copy(out=yq[:, 0:wq], in_=pyp[:, 0:wq])
                    nc.vector.tensor_mul(yfin[pi][:, 0:wq], yq[:, 0:wq],
                                         sz[pi][:, 0:wq])
                for h in range(NH):
                    da = T(sp, [128, WB], F16, "DA", 2, f"da{c}_{i}_{h}")
                    da3 = r3(da)
                    if da_zeroed[0] < 2:
                        # full-tile zero once per buffer: boundary cols stay 0
                        # and never-written tail cols stay finite
                        nc.vector.memset(da[:], 0.0)
                        da_zeroed[0] += 1
                    for nl in range(NPH):
                        n = h * NPH + nl
                        nc.scalar.activation(
                            out=da3[:, nl:nl + 1, 1:w + 1],
                            in_=deltas[i][:, 0:w], func=AF.Exp,
                            scale=An[i][:, n:n + 1])
                    dbu = T(sp, [128, WB], F16, "DBU", 1, f"dbu{c}_{i}_{h}")
                    dbu3 = r3(dbu)
                    if not dbu_zeroed[0]:
                        nc.vector.memset(dbu[:], 0.0)
                        dbu_zeroed[0] = True
                    if c > 0:
                        nc.vector.tensor_copy(
                            out=dbu3[:, :, 0:1],
                            in_=states[i][:, h * NPH:(h + 1) * NPH])
                    du_b = du[:, 0:w].unsqueeze(1).broadcast_to([128, NPH, w])
                    nc.vector.tensor_tensor(out=dbu3[:, :, 1:w + 1],
                                            in0=du_b,
                                            in1=r3(ball[h])[:, :, 1:w + 1],
                                            op=OP.mult)
                    ht = T(sp, [128, WB], F16, "H", 2, f"h{c}_{i}_{h}")
                    nc.vector.tensor_tensor_scan(ht[:], da[:], dbu[:], 0.0,
                                                 OP.mult, OP.add)
                    h3 = r3(ht)
                    if c < NC - 1:
                        nc.vector.tensor_copy(
                            out=states[i][:, h * NPH:(h + 1) * NPH],
                            in_=h3[:, :, w:w + 1])
                    hc = T(sp, [128, NPH * WMAX], F16, "HC", 2,
                           f"hc{c}_{i}_{h}")
                    hc3 = hc[:].rearrange("p (n q) -> p n q", n=NPH)
                    nc.vector.tensor_tensor(out=hc3[:, :, 0:w],
                                            in0=h3[:, :, 1:w + 1],
                                            in1=r3(call_[h])[:, :, 1:w + 1],
                                            op=OP.mult)
                    for nl in range(NPH):
                        nc.tensor.matmul(yp[:, 0:w], ident[:],
                                         hc[:, nl * WMAX: nl * WMAX + w],
                                         start=False,
                                         stop=(h == NH - 1 and nl == NPH - 1))
                pend.append((i, yp, w))
            pi, pyp, wq = pend.pop()
            yq = T(sp, [128, WMAX], F16, "yq", 2, f"yq{pi}_{c}")
            nc.scalar.copy(out=yq[:, 0:wq], in_=pyp[:, 0:wq])
            nc.vector.tensor_mul(yfin[pi][:, 0:wq], yq[:, 0:wq],
                                 sz[pi][:, 0:wq])

            # ================= out_proj + final proj =================
            yo = [T(st, [128, WMAX], F16, "yo", 4, f"yo{cb}_{c}")
                  for cb in range(NCB)]
            for cb in range(NCB):
                ps_o = T(pp, [128, WMAX], F32, "y", 2, f"ps_o{cb}_{c}")
                for wv in range(2):
                    oww = []
                    for j in range(4):
                        i = wv * 4 + j
                        w_ = T(st, [128, C], F16, "ow", 2, f"outw{c}_{cb}_{i}")
                        dma(out=w_[:], in_=d_outw[i])
                        oww.append(w_)
                    for j in range(4):
                        i = wv * 4 + j
                        nc.tensor.matmul(
                            ps_o[:, 0:w], oww[j][:, cb * 128:(cb + 1) * 128],
                            yfin[i][:, 0:w],
                            start=(i == 0), stop=(i == NDB - 1))
                nc.scalar.copy(out=yo[cb][:, 0:w], in_=ps_o[:, 0:w])
            for cb in range(NCB):
                ps_p = T(pp, [128, WMAX], F32, "y", 2, f"ps_p{cb}_{c}")
                for cib in range(NCB):
                    nc.tensor.matmul(ps_p[:, 0:w],
                                     pw[cib][:, cb * 128:(cb + 1) * 128],
                                     yo[cib][:, 0:w],
                                     start=(cib == 0), stop=(cib == NCB - 1))
                ot = T(hp, [128, WMAX], F32, "osb", 2, f"osb{cb}_{c}")
                nc.scalar.copy(out=ot[:, 0:w], in_=ps_p[:, 0:w])
                dma(out=d_part[cb * 128:(cb + 1) * 128, s:e],
                    in_=ot[:, 0:w])

            ub_prev = ub

    nc.compile()
    return nc


_cache = {}


def _prep_core_inputs(inputs, core):
    b = core >> 1
    rev = (core & 1) == 1
    p = "b_" if rev else "f_"
    f16 = np.float16
    f32 = np.float32

    toks = np.asarray(inputs["x"][b]).astype(np.int64)
    if rev:
        toks = toks[::-1]
    embf = np.asarray(inputs["emb"]).astype(f16)
    h0 = np.ascontiguousarray(embf[toks].T)

    key = ("wts", p)
    if key not in _cache:

        cw = np.asarray(inputs["conv_w"]).astype(f32)  # [D, cout, cin, K]
        if rev:
            cw = cw[:, :, :, ::-1]
        convw = np.empty((DEPTH, NCB, 128, K * NCB, 128), f16)
        for l in range(DEPTH):
            for cib in range(NCB):
                for k in range(K):
                    for cob in range(NCB):
                        blk = cw[l, cob * 128:(cob + 1) * 128,
                                 cib * 128:(cib + 1) * 128, k]
                        convw[l, cib, :, k * NCB + cob, :] = blk.T.astype(f16)
        cpar = np.zeros((DEPTH, 128, 12), f32)
        for l in range(DEPTH):
            for cob in range(NCB):
                cs = slice(cob * 128, (cob + 1) * 128)
                cpar[l, :, 0 * NCB + cob] = inputs["conv_b"][l][cs]
                cpar[l, :, 1 * NCB + cob] = inputs["ln_g"][l][cs]
                cpar[l, :, 2 * NCB + cob] = inputs["ln_b"][l][cs]

        in_w = np.asarray(inputs[p + "in_w"]).astype(f32)  # [2*DI, C]
        inw = np.empty((NCB, 128, 2 * DI), f16)
        for cib in range(NCB):
            inw[cib] = in_w[:, cib * 128:(cib + 1) * 128].T.astype(f16)

        mconv = np.asarray(inputs[p + "conv_w"]).astype(f32)  # [DI, 4]
        mcw = np.zeros((NDB, 128, DCONV * 128), f16)
        dd = np.arange(128)
        for i in range(NDB):
            for k in range(DCONV):
                mcw[i, dd, k * 128 + dd] = mconv[i * 128:(i + 1) * 128, k]

        mpar = np.zeros((128, 16), f32)
        for i in range(NDB):
            mpar[:, i] = inputs[p + "conv_b"][i * 128:(i + 1) * 128]
            mpar[:, 8 + i] = inputs[p + "dt_b"][i * 128:(i + 1) * 128]

        x_w = np.asarray(inputs[p + "x_w"]).astype(f32)  # [64, DI]
        xw = np.empty((NDB, 128, DTR + 2 * N), f16)
        for i in range(NDB):
            xw[i] = x_w[:, i * 128:(i + 1) * 128].T.astype(f16)

        dt_w = np.asarray(inputs[p + "dt_w"]).astype(f32)  # [DI, DTR]
        dtw = np.empty((NDB, DTR, 128), f16)
        for i in range(NDB):
            dtw[i] = dt_w[i * 128:(i + 1) * 128, :].T.astype(f16)

        An = (-np.exp(np.asarray(inputs[p + "A_log"]).astype(f32))
              ).reshape(NDB, 128, N).astype(f32)

        Dv = np.asarray(inputs[p + "D"]).astype(f32)
        Dd = np.zeros((NDB, 128, 128), f16)
        for i in range(NDB):
            Dd[i, dd, dd] = Dv[i * 128:(i + 1) * 128]

        out_w = np.asarray(inputs[p + "out_w"]).astype(f32)  # [C, DI]
        outw = np.empty((NDB, 128, C), f16)
        for i in range(NDB):
            outw[i] = out_w[:, i * 128:(i + 1) * 128].T.astype(f16)

        proj_w = np.asarray(inputs["proj_w"]).astype(f32)  # [C, 2C]
        half = proj_w[:, C:] if rev else proj_w[:, :C]
        pw = np.empty((NCB, 128, C), f16)
        for cib in range(NCB):
            pw[cib] = half[:, cib * 128:(cib + 1) * 128].T.astype(f16)

        _cache[key] = dict(
            convw=convw, cpar=cpar, inw=inw, mcw=mcw, mpar=mpar,
            xw=xw, dtw=dtw, An=An, Dd=Dd, outw=outw, pw=pw,
            ident=np.eye(128, dtype=f16))
    m = dict(_cache[key])
    m["h0"] = h0
    return m


def kernel(**inputs):
    if "nc" not in _cache:
        _cache["nc"] = build_program()
    nc = _cache["nc"]
    for k in [k for k in _cache if k != "nc"]:
        del _cache[k]
    in_maps = [_prep_core_inputs(inputs, c) for c in range(8)]
    res = run_bass_kernel_spmd(nc, in_maps, list(range(8)))
    parts = [r["part"] for r in res.results]
    proj_b = np.asarray(inputs["proj_b"]).astype(np.float32)
    out = np.empty((B, L, C), np.float32)
    for b in range(B):
        # note: the reference concatenates bo still in reversed time order
        comb = parts[2 * b] + parts[2 * b + 1]
        out[b] = comb.T + proj_b[None, :]
    m = np.asarray(inputs["m"])
    out = np.where(m[:, :, None], 0.0, out).astype(np.float32)
    return out


# revision 13
# speedup vs baseline: 1.3129x; 1.0516x over previous
"""BiMambaTextEncoder Trainium2 kernel (chunk-pipelined).

Sharding: 8 cores = 4 batch x 2 direction; backward handled by reversing the
sequence on the host and flipping conv kernels; final projection decomposed so
the host sums fo/bo halves (no collectives).

This version pipelines the conv stack with the Mamba scan phase: everything is
emitted in NC time-chunks of Q columns.  Conv layer l's chunk c covers columns
[c*Q - 2*l, (c+1)*Q - 2*l) so a layer's chunk depends only on chunks 0..c of
the previous layer (the +-2 halo lands exactly on the previous chunk's end).
Downstream (in_proj, dwconv, x_proj, dt, scan, out_proj) uses layer-2's chunk
ranges.  The Tile scheduler then overlaps chunk c's scan work (Vector-bound)
with chunk c+1's conv work (PE-bound), which removes the ~200us of Vector idle
the phase-sequential version had.

Scan layout per (i-block, n-half, chunk): one [128, 8*261] tensor_tensor_scan;
column n*261 is a boundary column holding dA=0 and dBu=carry-state, restarting
the recurrence per state.  DA/DBU tiles are fully memset once per buffer so
never-written tail columns (chunks narrower than 260) stay finite (a NaN there
would poison the boundary restart via 0*NaN).
"""

from contextlib import ExitStack

import numpy as np

import concourse.bass as bass
from concourse import bacc
import concourse.mybir as mybir
import concourse.tile as tile
from concourse.bass_utils import run_bass_kernel_spmd

F16 = mybir.dt.float16
F32 = mybir.dt.float32
AF = mybir.ActivationFunctionType
OP = mybir.AluOpType

B, L, C, K, DEPTH, V = 4, 1024, 512, 5, 3, 178
DI = 1024           # d_inner
N = 16              # d_state
DCONV = 4
DTR = 32            # dt_rank
NCB = C // 128      # 4 channel blocks
NDB = DI // 128     # 8 d_inner blocks
Q = 256             # chunk width
NC = L // Q         # chunks
EPS = 1e-5

NH = 2              # n halves per block
NPH = N // NH       # 8 states per packed scan
WMAX = Q + 4        # widest chunk (last downstream chunk)
SB = WMAX + 1       # 261: per-state stride in packed scan tiles
WB = NPH * SB       # packed scan width


def _ranges(shift):
    rs = []
    for c in range(NC):
        s = max(0, c * Q - shift)
        e = L if c == NC - 1 else (c + 1) * Q - shift
        rs.append((s, e))
    return rs


def _par(param, cob):
    s = param * NCB + cob
    return slice(s, s + 1)


def build_program():
    nc = bacc.Bacc()

    d_h0 = nc.dram_tensor("h0", [C, L], F16, kind="ExternalInput")
    d_convw = nc.dram_tensor("convw", [DEPTH, NCB, 128, K * NCB, 128], F16,
                             kind="ExternalInput")
    d_cpar = nc.dram_tensor("cpar", [DEPTH, 128, 12], F32, kind="ExternalInput")
    d_inw = nc.dram_tensor("inw", [NCB, 128, 2 * DI], F16, kind="ExternalInput")
    d_mcw = nc.dram_tensor("mcw", [NDB, 128, DCONV * 128], F16,
                           kind="ExternalInput")
    d_mpar = nc.dram_tensor("mpar", [128, 16], F32, kind="ExternalInput")
    d_xw = nc.dram_tensor("xw", [NDB, 128, DTR + 2 * N], F16,
                          kind="ExternalInput")
    d_dtw = nc.dram_tensor("dtw", [NDB, DTR, 128], F16, kind="ExternalInput")
    d_An = nc.dram_tensor("An", [NDB, 128, N], F32, kind="ExternalInput")
    d_Dd = nc.dram_tensor("Dd", [NDB, 128, 128], F16, kind="ExternalInput")
    d_outw = nc.dram_tensor("outw", [NDB, 128, C], F16, kind="ExternalInput")
    d_pw = nc.dram_tensor("pw", [NCB, 128, C], F16, kind="ExternalInput")
    d_ident = nc.dram_tensor("ident", [128, 128], F16, kind="ExternalInput")
    d_part = nc.dram_tensor("part", [C, L], F32, kind="ExternalOutput")
    # DRAM bounce for B/C rows of x_proj output (for broadcast reads)
    d_xbc = nc.dram_tensor("xbc", [2 * N, L], F16)

    CR = [_ranges(2 * l) for l in range(DEPTH)]
    MR = CR[DEPTH - 1]

    with tile.TileContext(nc) as tc, ExitStack() as ctx:
        sing = ctx.enter_context(tc.tile_pool(name="sing", bufs=1))
        wp = ctx.enter_context(tc.tile_pool(name="wp", bufs=1))
        hp = ctx.enter_context(tc.tile_pool(name="hp", bufs=1))
        bcp = ctx.enter_context(tc.tile_pool(name="bcp", bufs=1))
        sp = ctx.enter_context(tc.tile_pool(name="sp", bufs=1))
        st = ctx.enter_context(tc.tile_pool(name="st", bufs=1))
        pp = ctx.enter_context(tc.tile_pool(name="pp", bufs=1, space="PSUM"))

        dma = nc.sync.dma_start

        def T(pool, shape, dt, tag, bufs, name):
            return pool.tile(shape, dt, tag=tag, bufs=bufs, name=name)

        def r3(t):
            return t[:].rearrange("p (n q) -> p n q", n=NPH)

        # ---- constants / params ----
        ident = T(sing, [128, 128], F16, "ident", 1, "ident")
        dma(out=ident[:], in_=d_ident[:])
        ones = T(sing, [128, 1], F16, "ones", 1, "ones")
        nc.vector.memset(ones[:], 1.0)
        ones32 = T(sing, [128, 1], F32, "ones32", 1, "ones32")
        nc.vector.memset(ones32[:], 1.0)
        epst = T(sing, [1, 1], F32, "epst", 1, "epst")
        nc.vector.memset(epst[:], EPS)
        cpar = []
        for l in range(DEPTH):
            t = T(sing, [128, 12], F32, f"cpar{l}", 1, f"cpar{l}")
            dma(out=t[:], in_=d_cpar[l])
            cpar.append(t)
        mpar = T(sing, [128, 16], F32, "mpar", 1, "mpar")
        dma(out=mpar[:], in_=d_mpar[:])
        An = []
        for i in range(NDB):
            t = T(sing, [128, N], F32, f"An{i}", 1, f"An{i}")
            dma(out=t[:], in_=d_An[i])
            An.append(t)
        states = []
        for i in range(NDB):
            t = T(sing, [128, N], F16, f"stt{i}", 1, f"stt{i}")
            states.append(t)

        # pre-touch every ACT-consumed param tile on the scalar engine so the
        # real consumers don't exceed the Activation ISA sync-wait limit
        touch = T(sing, [128, 224], F16, "touch", 1, "touch")
        for ti_, tt_ in enumerate(cpar + [mpar] + An):
            w_ = tt_.shape[-1]
            nc.scalar.copy(out=touch[:, ti_ * 16: ti_ * 16 + w_], in_=tt_[:])
        nc.scalar.copy(out=touch[0:1, 223:224], in_=epst[:])

        LP = L + 4
        hbuf = [[T(hp, [128, LP], F16, "big", 8, f"hbuf{s}_{cb}")
                 for cb in range(NCB)] for s in range(2)]
        for s in range(2):
            for cb in range(NCB):
                nc.vector.memset(hbuf[s][cb][:, 0:2], 0.0)
                nc.vector.memset(hbuf[s][cb][:, L + 2:LP], 0.0)

        for cb in range(NCB):
            dma(out=hbuf[0][cb][:, 2:2 + L],
                in_=d_h0[cb * 128:(cb + 1) * 128, :])

        # ---- resident weights ----
        cw = [[None] * NCB for _ in range(DEPTH)]
        for l in range(DEPTH):
            for cib in range(NCB):
                t = T(wp, [128, K * NCB * 128], F16, "cw", 12, f"cw{l}_{cib}")
                dma(out=t[:], in_=d_convw[l, cib])
                cw[l][cib] = t
        inw = []
        for cib in range(NCB):
            t = T(wp, [128, 2 * DI], F16, "inw", 4, f"inw{cib}")
            dma(out=t[:], in_=d_inw[cib])
            inw.append(t)
        mcw = []
        for i in range(NDB):
            t = T(wp, [128, DCONV * 128], F16, "mcw", 8, f"mcw{i}")
            dma(out=t[:], in_=d_mcw[i])
            mcw.append(t)
        xw = []
        for i in range(NDB):
            t = T(wp, [128, DTR + 2 * N], F16, f"xw{i}", 1, f"xw{i}")
            dma(out=t[:], in_=d_xw[i])
            xw.append(t)
        dtw = []
        for i in range(NDB):
            t = T(wp, [DTR, 128], F16, f"dtw{i}", 1, f"dtw{i}")
            dma(out=t[:], in_=d_dtw[i])
            dtw.append(t)
        Dd = []
        for i in range(NDB):
            t = T(wp, [128, 128], F16, f"Dd{i}", 1, f"Dd{i}")
            dma(out=t[:], in_=d_Dd[i])
            Dd.append(t)
        pw = []
        for cib in range(NCB):
            t = T(wp, [128, C], F16, "pw", 4, f"pw{cib}")
            dma(out=t[:], in_=d_pw[cib])
            pw.append(t)

        xdbc = T(hp, [DTR + 2 * N, L], F16, "xdbc", 1, "xdbc")

        FR = {}             # per-chunk front tiles
        misc = {"dbu_zeroed": False, "da_zeroed": 0}

        def emit_conv_piece(c, l, cobs, do_ln):
            s, e = CR[l][c]
            w = e - s
            src_ = hbuf[l % 2]
            dst = hbuf[(l + 1) % 2]
            key = ("craw", c, l)
            if key not in FR:
                FR[key] = [T(st, [128, WMAX], F16, "craw", 6,
                             f"craw{l}_{cob}_{c}") for cob in range(NCB)]
            craw = FR[key]
            for cob in cobs:
                ps = T(pp, [128, WMAX], F32, "mm", 2, f"ps_c{l}_{cob}_{c}")
                first = True
                for cib in range(NCB):
                    for k in range(K):
                        j = k * NCB + cob
                        nc.tensor.matmul(
                            ps[:, 0:w],
                            cw[l][cib][:, j * 128:(j + 1) * 128],
                            src_[cib][:, s + k: s + k + w],
                            start=first,
                            stop=(cib == NCB - 1 and k == K - 1))
                        first = False
                nc.scalar.activation(
                    out=craw[cob][:, 0:w], in_=ps[:, 0:w],
                    func=AF.Identity, bias=cpar[l][:, _par(0, cob)],
                    scale=1.0)
            if not do_ln:
                return
            ps_s = T(pp, [1, WMAX], F32, "st", 2, f"ps_s{l}_{c}")
            ps_q = T(pp, [1, WMAX], F32, "st", 2, f"ps_q{l}_{c}")
            for cob in range(NCB):
                nc.tensor.matmul(ps_s[:, 0:w], ones[:],
                                 craw[cob][:, 0:w],
                                 start=(cob == 0), stop=(cob == NCB - 1))
            for cob in range(NCB):
                sq = T(st, [128, WMAX], F16, "csq", 1, f"csq{l}_{cob}_{c}")
                nc.scalar.activation(out=sq[:, 0:w],
                                     in_=craw[cob][:, 0:w],
                                     func=AF.Square)
                nc.tensor.matmul(ps_q[:, 0:w], ones[:], sq[:, 0:w],
                                 start=(cob == 0), stop=(cob == NCB - 1))
            mu = T(st, [1, WMAX], F32, "row", 3, f"mu{l}_{c}")
            nc.vector.tensor_scalar_mul(mu[:, 0:w], ps_s[:, 0:w], 1.0 / C)
            var = T(st, [1, WMAX], F32, "row", 3, f"var{l}_{c}")
            nc.vector.tensor_mul(var[:, 0:w], mu[:, 0:w], mu[:, 0:w])
            nc.vector.tensor_scalar_mul(var[:, 0:w], var[:, 0:w], -1.0)
            nc.vector.scalar_tensor_tensor(
                out=var[:, 0:w], in0=ps_q[:, 0:w], scalar=1.0 / C,
                in1=var[:, 0:w], op0=OP.mult, op1=OP.add)
            nc.scalar.activation(out=var[:, 0:w], in_=var[:, 0:w],
                                 func=AF.Sqrt, bias=epst[:], scale=1.0)
            rstd = T(st, [1, WMAX], F32, "row", 3, f"rstd{l}_{c}")
            nc.vector.reciprocal_approx_fast(out=rstd[:, 0:w],
                                             in_=var[:, 0:w])
            nmr = T(st, [1, WMAX], F16, "row16", 2, f"nmr{l}_{c}")
            nc.vector.tensor_mul(nmr[:, 0:w], mu[:, 0:w], rstd[:, 0:w])
            nc.vector.tensor_scalar_mul(nmr[:, 0:w], nmr[:, 0:w], -1.0)
            rstd16 = T(st, [1, WMAX], F16, "row16", 2, f"rstd16{l}_{c}")
            nc.vector.tensor_copy(out=rstd16[:, 0:w], in_=rstd[:, 0:w])
            rs_bc = T(st, [128, WMAX], F16, "rs_bc", 2, f"rs_bc{l}_{c}")
            nc.gpsimd.partition_broadcast(rs_bc[:, 0:w], rstd16[:, 0:w])
            nm_bc = T(st, [128, WMAX], F16, "nm_bc", 2, f"nm_bc{l}_{c}")
            nc.gpsimd.partition_broadcast(nm_bc[:, 0:w], nmr[:, 0:w])
            for cob in range(NCB):
                t2 = T(st, [128, WMAX], F16, "lnt", 2, f"lnt{l}_{cob}_{c}")
                nc.vector.tensor_mul(t2[:, 0:w], craw[cob][:, 0:w],
                                     rs_bc[:, 0:w])
                nc.vector.tensor_add(t2[:, 0:w], t2[:, 0:w],
                                     nm_bc[:, 0:w])
                nc.scalar.activation(
                    out=dst[cob][:, 2 + s: 2 + e],
                    in_=t2[:, 0:w], func=AF.Prelu,
                    bias=cpar[l][:, _par(2, cob)],
                    scale=cpar[l][:, _par(1, cob)], alpha=0.2)

        def emit_inproj_u(c):
            hfin = hbuf[DEPTH % 2]
            s, e = MR[c]
            w = e - s
            ub = [T(hp, [128, 3 + WMAX], F16, "ub", 12, f"ub{i}_{c}")
                  for i in range(NDB)]
            FR[("ub", c)] = ub
            for i in range(NDB):
                if c == 0:
                    nc.vector.memset(ub[i][:, 0:3], 0.0)
                else:
                    ub_prev = FR[("ub", c - 1)]
                    pw_ = MR[c - 1][1] - MR[c - 1][0]
                    nc.scalar.copy(out=ub[i][:, 0:3],
                                   in_=ub_prev[i][:, pw_:pw_ + 3])
            for m in range(NDB):
                ps = T(pp, [128, WMAX], F32, "mp", 2, f"ps_in{m}_{c}")
                for cib in range(NCB):
                    nc.tensor.matmul(
                        ps[:, 0:w], inw[cib][:, m * 128:(m + 1) * 128],
                        hfin[cib][:, 2 + s: 2 + e],
                        start=(cib == 0), stop=(cib == NCB - 1))
                nc.scalar.copy(out=ub[m][:, 3:3 + w], in_=ps[:, 0:w])

        def emit_mconv_xproj(c):
            s, e = MR[c]
            w = e - s
            ub = FR[("ub", c)]
            uc = [T(hp, [128, WMAX], F16, "uc", 12, f"uc{i}_{c}")
                  for i in range(NDB)]
            FR[("uc", c)] = uc
            for i in range(NDB):
                ps = T(pp, [128, WMAX], F32, "mp", 2, f"ps_mc{i}_{c}")
                for k in range(DCONV):
                    nc.tensor.matmul(
                        ps[:, 0:w], mcw[i][:, k * 128:(k + 1) * 128],
                        ub[i][:, k: k + w],
                        start=(k == 0), stop=(k == DCONV - 1))
                nc.scalar.activation(
                    out=uc[i][:, 0:w], in_=ps[:, 0:w],
                    func=AF.Silu, bias=mpar[:, i:i + 1], scale=1.0)
            ps = T(pp, [DTR + 2 * N, WMAX], F32, "mp", 2, f"ps_x{c}")
            for i in range(NDB):
                nc.tensor.matmul(ps[:, 0:w], xw[i][:], uc[i][:, 0:w],
                                 start=(i == 0), stop=(i == NDB - 1))
            nc.scalar.copy(out=xdbc[:, s:e], in_=ps[:, 0:w])
            dma(out=d_xbc[:, s:e], in_=xdbc[DTR:DTR + 2 * N, s:e])

        def emit_z_dt_bc(c):
            hfin = hbuf[DEPTH % 2]
            s, e = MR[c]
            w = e - s
            sz = [T(hp, [128, WMAX], F16, "sz", 12, f"sz{i}_{c}")
                  for i in range(NDB)]
            FR[("sz", c)] = sz
            for m in range(NDB, 2 * NDB):
                ps = T(pp, [128, WMAX], F32, "mp", 2, f"ps_in{m}_{c}")
                for cib in range(NCB):
                    nc.tensor.matmul(
                        ps[:, 0:w], inw[cib][:, m * 128:(m + 1) * 128],
                        hfin[cib][:, 2 + s: 2 + e],
                        start=(cib == 0), stop=(cib == NCB - 1))
                nc.scalar.activation(out=sz[m - NDB][:, 0:w], in_=ps[:, 0:w],
                                     func=AF.Silu)
            deltas = []
            for i in range(NDB):
                ps = T(pp, [128, WMAX], F32, "mp", 2, f"ps_dt{i}_{c}")
                nc.tensor.matmul(ps[:, 0:w], dtw[i][:], xdbc[0:DTR, s:e],
                                 start=True, stop=True)
                dl = T(sp, [128, WMAX], F16, "delta", 10, f"delta{i}_{c}")
                nc.scalar.activation(out=dl[:, 0:w], in_=ps[:, 0:w],
                                     func=AF.Exp,
                                     bias=mpar[:, 8 + i:9 + i], scale=1.0)
                deltas.append(dl)
            for i in range(NDB):
                nc.scalar.activation(out=deltas[i][:, 0:w],
                                     in_=deltas[i][:, 0:w],
                                     func=AF.Ln, bias=ones32[:], scale=1.0)
            FR[("deltas", c)] = deltas
            ball, call_ = [], []
            for h in range(NH):
                bt = T(bcp, [128, WB], F16, "ball", 2, f"ball{c}_{h}")
                for p0 in (0, 64):
                    dma(out=r3(bt)[p0:p0 + 64, :, 1:w + 1],
                        in_=d_xbc[NPH * h:NPH * h + NPH,
                                  s:e].partition_broadcast(64))
                ball.append(bt)
                ct = T(bcp, [128, WB], F16, "call", 2, f"call{c}_{h}")
                for p0 in (0, 64):
                    dma(out=r3(ct)[p0:p0 + 64, :, 1:w + 1],
                        in_=d_xbc[N + NPH * h:N + NPH * h + NPH,
                                  s:e].partition_broadcast(64))
                call_.append(ct)
            FR[("ball", c)] = ball
            FR[("call", c)] = call_

        def pieces(c):
            if c >= NC:
                return []
            return [
                lambda: emit_conv_piece(c, 0, (0, 1, 2, 3), True),
                lambda: emit_conv_piece(c, 1, (0, 1), False),
                lambda: emit_conv_piece(c, 1, (2, 3), True),
                lambda: emit_conv_piece(c, 2, (0, 1), False),
                lambda: emit_conv_piece(c, 2, (2, 3), True),
                lambda: emit_inproj_u(c),
                lambda: emit_mconv_xproj(c),
                lambda: emit_z_dt_bc(c),
            ]

        def emit_scan_block(c, fill):
            s, e = MR[c]
            w = e - s
            uc = FR[("uc", c)]
            sz = FR[("sz", c)]
            deltas = FR[("deltas", c)]
            ball = FR[("ball", c)]
            call_ = FR[("call", c)]
            yfin = [T(hp, [128, WMAX], F16, "yf", 10, f"yf{i}_{c}")
                    for i in range(NDB)]
            FR[("yfin", c)] = yfin
            pend = []
            for i in range(NDB):
                yp = T(pp, [128, WMAX], F32, "y", 2, f"yp{i}_{c}")
                nc.tensor.matmul(yp[:, 0:w], Dd[i][:], uc[i][:, 0:w],
                                 start=True, stop=False)
                du = T(sp, [128, WMAX], F16, "du", 2, f"du{i}_{c}")
                nc.vector.tensor_mul(du[:, 0:w], deltas[i][:, 0:w],
                                     uc[i][:, 0:w])
                if pend:
                    pi, pyp, wq = pend.pop()
                    yq = T(sp, [128, WMAX], F16, "yq", 2, f"yq{pi}_{c}")
                    nc.scalar.copy(out=yq[:, 0:wq], in_=pyp[:, 0:wq])
                    nc.vector.tensor_mul(yfin[pi][:, 0:wq], yq[:, 0:wq],
                                         sz[pi][:, 0:wq])
                for h in range(NH):
                    da = T(sp, [128, WB], F16, "DA", 2, f"da{c}_{i}_{h}")
                    da3 = r3(da)
                    if misc["da_zeroed"] < 2:
                        # full-tile zero once per buffer: boundary cols stay
                        # 0 and never-written tail cols stay finite
                        nc.vector.memset(da[:], 0.0)
                        misc["da_zeroed"] += 1
                    for nl in range(NPH):
                        n = h * NPH + nl
                        nc.scalar.activation(
                            out=da3[:, nl:nl + 1, 1:w + 1],
                            in_=deltas[i][:, 0:w], func=AF.Exp,
                            scale=An[i][:, n:n + 1])
                    dbu = T(sp, [128, WB], F16, "DBU", 1, f"dbu{c}_{i}_{h}")
                    dbu3 = r3(dbu)
                    if not misc["dbu_zeroed"]:
                        nc.vector.memset(dbu[:], 0.0)
                        misc["dbu_zeroed"] = True
                    if c > 0:
                        nc.vector.tensor_copy(
                            out=dbu3[:, :, 0:1],
                            in_=states[i][:, h * NPH:(h + 1) * NPH])
                    du_b = du[:, 0:w].unsqueeze(1).broadcast_to(
                        [128, NPH, w])
                    nc.vector.tensor_tensor(out=dbu3[:, :, 1:w + 1],
                                            in0=du_b,
                                            in1=r3(ball[h])[:, :, 1:w + 1],
                                            op=OP.mult)
                    ht = T(sp, [128, WB], F16, "H", 2, f"h{c}_{i}_{h}")
                    nc.vector.tensor_tensor_scan(ht[:], da[:], dbu[:], 0.0,
                                                 OP.mult, OP.add)
                    h3 = r3(ht)
                    if c < NC - 1:
                        nc.vector.tensor_copy(
                            out=states[i][:, h * NPH:(h + 1) * NPH],
                            in_=h3[:, :, w:w + 1])
                    hc = T(sp, [128, NPH * WMAX], F16, "HC", 2,
                           f"hc{c}_{i}_{h}")
                    hc3 = hc[:].rearrange("p (n q) -> p n q", n=NPH)
                    nc.vector.tensor_tensor(out=hc3[:, :, 0:w],
                                            in0=h3[:, :, 1:w + 1],
                                            in1=r3(call_[h])[:, :, 1:w + 1],
                                            op=OP.mult)
                    for nl in range(NPH):
                        nc.tensor.matmul(yp[:, 0:w], ident[:],
                                         hc[:, nl * WMAX: nl * WMAX + w],
                                         start=False,
                                         stop=(h == NH - 1 and nl == NPH - 1))
                pend.append((i, yp, w))
                if i < len(fill):
                    fill[i]()
            pi, pyp, wq = pend.pop()
            yq = T(sp, [128, WMAX], F16, "yq", 2, f"yq{pi}_{c}")
            nc.scalar.copy(out=yq[:, 0:wq], in_=pyp[:, 0:wq])
            nc.vector.tensor_mul(yfin[pi][:, 0:wq], yq[:, 0:wq],
                                 sz[pi][:, 0:wq])

        def emit_outproj(c):
            s, e = MR[c]
            w = e - s
            yfin = FR[("yfin", c)]
            yo = [T(st, [128, WMAX], F16, "yo", 4, f"yo{cb}_{c}")
                  for cb in range(NCB)]
            for cb in range(NCB):
                ps_o = T(pp, [128, WMAX], F32, "y", 2, f"ps_o{cb}_{c}")
                for wv in range(2):
                    oww = []
                    for j in range(4):
                        i = wv * 4 + j
                        w_ = T(st, [128, C], F16, "ow", 2, f"outw{c}_{cb}_{i}")
                        dma(out=w_[:], in_=d_outw[i])
                        oww.append(w_)
                    for j in range(4):
                        i = wv * 4 + j
                        nc.tensor.matmul(
                            ps_o[:, 0:w], oww[j][:, cb * 128:(cb + 1) * 128],
                            yfin[i][:, 0:w],
                            start=(i == 0), stop=(i == NDB - 1))
                nc.scalar.copy(out=yo[cb][:, 0:w], in_=ps_o[:, 0:w])
            for cb in range(NCB):
                ps_p = T(pp, [128, WMAX], F32, "y", 2, f"ps_p{cb}_{c}")
                for cib in range(NCB):
                    nc.tensor.matmul(ps_p[:, 0:w],
                                     pw[cib][:, cb * 128:(cb + 1) * 128],
                                     yo[cib][:, 0:w],
                                     start=(cib == 0), stop=(cib == NCB - 1))
                ot = T(hp, [128, WMAX], F32, "osb", 2, f"osb{cb}_{c}")
                nc.scalar.copy(out=ot[:, 0:w], in_=ps_p[:, 0:w])
                dma(out=d_part[cb * 128:(cb + 1) * 128, s:e],
                    in_=ot[:, 0:w])

        for p in pieces(0):
            p()
        for c in range(NC):
            emit_scan_block(c, pieces(c + 1))
            emit_outproj(c)

    nc.compile()
    return nc


_cache = {}


def _prep_core_inputs(inputs, core):
    b = core >> 1
    rev = (core & 1) == 1
    p = "b_" if rev else "f_"
    f16 = np.float16
    f32 = np.float32

    toks = np.asarray(inputs["x"][b]).astype(np.int64)
    if rev:
        toks = toks[::-1]
    embf = np.asarray(inputs["emb"]).astype(f16)
    h0 = np.ascontiguousarray(embf[toks].T)

    key = ("wts", p)
    if key not in _cache:

        cw = np.asarray(inputs["conv_w"]).astype(f32)  # [D, cout, cin, K]
        if rev:
            cw = cw[:, :, :, ::-1]
        convw = np.empty((DEPTH, NCB, 128, K * NCB, 128), f16)
        for l in range(DEPTH):
            for cib in range(NCB):
                for k in range(K):
                    for cob in range(NCB):
                        blk = cw[l, cob * 128:(cob + 1) * 128,
                                 cib * 128:(cib + 1) * 128, k]
                        convw[l, cib, :, k * NCB + cob, :] = blk.T.astype(f16)
        cpar = np.zeros((DEPTH, 128, 12), f32)
        for l in range(DEPTH):
            for cob in range(NCB):
                cs = slice(cob * 128, (cob + 1) * 128)
                cpar[l, :, 0 * NCB + cob] = inputs["conv_b"][l][cs]
                cpar[l, :, 1 * NCB + cob] = inputs["ln_g"][l][cs]
                cpar[l, :, 2 * NCB + cob] = inputs["ln_b"][l][cs]

        in_w = np.asarray(inputs[p + "in_w"]).astype(f32)  # [2*DI, C]
        inw = np.empty((NCB, 128, 2 * DI), f16)
        for cib in range(NCB):
            inw[cib] = in_w[:, cib * 128:(cib + 1) * 128].T.astype(f16)

        mconv = np.asarray(inputs[p + "conv_w"]).astype(f32)  # [DI, 4]
        mcw = np.zeros((NDB, 128, DCONV * 128), f16)
        dd = np.arange(128)
        for i in range(NDB):
            for k in range(DCONV):
                mcw[i, dd, k * 128 + dd] = mconv[i * 128:(i + 1) * 128, k]

        mpar = np.zeros((128, 16), f32)
        for i in range(NDB):
            mpar[:, i] = inputs[p + "conv_b"][i * 128:(i + 1) * 128]
            mpar[:, 8 + i] = inputs[p + "dt_b"][i * 128:(i + 1) * 128]

        x_w = np.asarray(inputs[p + "x_w"]).astype(f32)  # [64, DI]
        xw = np.empty((NDB, 128, DTR + 2 * N), f16)
        for i in range(NDB):
            xw[i] = x_w[:, i * 128:(i + 1) * 128].T.astype(f16)

        dt_w = np.asarray(inputs[p + "dt_w"]).astype(f32)  # [DI, DTR]
        dtw = np.empty((NDB, DTR, 128), f16)
        for i in range(NDB):
            dtw[i] = dt_w[i * 128:(i + 1) * 128, :].T.astype(f16)

        An = (-np.exp(np.asarray(inputs[p + "A_log"]).astype(f32))
              ).reshape(NDB, 128, N).astype(f32)

        Dv = np.asarray(inputs[p + "D"]).astype(f32)
        Dd = np.zeros((NDB, 128, 128), f16)
        for i in range(NDB):
            Dd[i, dd, dd] = Dv[i * 128:(i + 1) * 128]

        out_w = np.asarray(inputs[p + "out_w"]).astype(f32)  # [C, DI]
        outw = np.empty((NDB, 128, C), f16)
        for i in range(NDB):
            outw[i] = out_w[:, i * 128:(i + 1) * 128].T.astype(f16)

        proj_w = np.asarray(inputs["proj_w"]).astype(f32)  # [C, 2C]
        half = proj_w[:, C:] if rev else proj_w[:, :C]
        pw = np.empty((NCB, 128, C), f16)
        for cib in range(NCB):
            pw[cib] = half[:, cib * 128:(cib + 1) * 128].T.astype(f16)

        _cache[key] = dict(
            convw=convw, cpar=cpar, inw=inw, mcw=mcw, mpar=mpar,
            xw=xw, dtw=dtw, An=An, Dd=Dd, outw=outw, pw=pw,
            ident=np.eye(128, dtype=f16))
    m = dict(_cache[key])
    m["h0"] = h0
    return m


def kernel(**inputs):
    if "nc" not in _cache:
        _cache["nc"] = build_program()
    nc = _cache["nc"]
    for k in [k for k in _cache if k != "nc"]:
        del _cache[k]
    in_maps = [_prep_core_inputs(inputs, c) for c in range(8)]
    res = run_bass_kernel_spmd(nc, in_maps, list(range(8)))
    parts = [r["part"] for r in res.results]
    proj_b = np.asarray(inputs["proj_b"]).astype(np.float32)
    out = np.empty((B, L, C), np.float32)
    for b in range(B):
        # note: the reference concatenates bo still in reversed time order
        comb = parts[2 * b] + parts[2 * b + 1]
        out[b] = comb.T + proj_b[None, :]
    m = np.asarray(inputs["m"])
    out = np.where(m[:, :, None], 0.0, out).astype(np.float32)
    return out
